# revision 9
# baseline (speedup 1.0000x reference)
"""DenseQConv1D Trainium2 kernel, v9 — closed-form ring-CNOT entangler.

Math (validated vs the jax reference): with y[b,l] = sum_c x[b,c,l]^2,
n2 = box8(y), z = alt8(y):
    out[b,c,t] = cos(theta[c,0]) * z[b,t] / n2[b,t]

Schedule (engine budgets balanced from the v5/v8 traces; DVE ops cost
~280ns flat, gpsimd ~2x that, each dma_start ~700ns of issue + ~650ns
DGE delay + ~400ns completion-semaphore):
  - gpsimd builds ONLY the 4-matmul filter mask W (its one long pole in
    v8 was 6 affine_selects + the cos chain, ~6us serial).
  - cos(theta) runs on the otherwise-idle ACT engine via the iterated
    affine-squares chain; the single ACT table (Square/Copy family) is
    prefetched by a dummy op during the input-DMA dead time.
  - theta rides pre-expanded as two spare columns of the input tiles
    (csEO[p,h] = cos(theta[2(p%8)+h]) per-partition scale for the final
    PSUM evacuation); the two evac scale-multiplies run in PARALLEL on
    vector and scalar.
  - the final-broadcast weights W64 arrive as an f32r DMA parameter on
    the idle second slot of the sync queue (no gpsimd/DVE time at all).
  - 1/n2 is the single-instruction custom-DVE reciprocal_approx_fast.
  - A8 (alt 8-tap) runs on gpsimd in parallel with B8/recip on vector.
"""

import numpy as np

B = 8
C_IN = 16
C_OUT = 16
L = 1024
K = 8
L_OUT = L - K + 1  # 1017
NCORE = 8
NOUT = 128          # output columns per core (last core uses 121)
NIN = NOUT + K - 1  # 135 input columns per core
NWIDE = 2 * NIN + 2  # 272: two channel halves + two theta columns

# cos(t) via iterated affine squares (max err ~3e-6 for |t| <= 2.9):
# g = (((t^2*c0+c1)^2*c2+c3)^2*c4+c5)^2; cos = c6*g + c7
CQ = [0.016203629623126187, -1.4420320349422868,
      -0.9882186745906292, 0.5363028899458568,
      -1.2270615909104523, 1.4530563055120675,
      1.0548677666154753, -0.9999652180452864]

_CACHE = {}


def _build_nc():
    import concourse.bacc as bacc
    import concourse.mybir as mybir
    import concourse.tile as tile

    f32 = mybir.dt.float32
    f32r = mybir.dt.float32r
    ALU = mybir.AluOpType
    AF = mybir.ActivationFunctionType

    nc = bacc.Bacc("TRN2", target_bir_lowering=False, debug=False)

    def act_raw(out, in_, func, bias=0.0, scale=1.0):
        eng = nc.scalar
        bias_arg = (
            eng.lower_ap(bias)
            if not isinstance(bias, float)
            else mybir.ImmediateValue(dtype=mybir.dt.float32, value=bias)
        )
        ins = [
            eng.lower_ap(in_),
            bias_arg,
            mybir.ImmediateValue(dtype=mybir.dt.float32, value=scale),
            mybir.ImmediateValue(dtype=mybir.dt.float32, value=0.0),
        ]
        return eng.add_instruction(
            mybir.InstActivation(
                name=nc.get_next_instruction_name(), func=func,
                ins=ins, outs=[eng.lower_ap(out)],
            )
        )

    xa_ext = nc.declare_dram_parameter("xa", [32, NWIDE], f32, isOutput=False)
    xb_ext = nc.declare_dram_parameter("xb", [32, NWIDE], f32, isOutput=False)
    w64_ext = nc.declare_dram_parameter("w64", [8, 64], f32r, isOutput=False)
    out_ext = nc.declare_dram_parameter("out", [64, 2 * NOUT], f32, isOutput=True)

    with tile.TileContext(nc) as tc, \
            tc.tile_pool(name="sb", bufs=1) as sb, \
            tc.tile_pool(name="ps", bufs=1, space="PSUM") as psp:
        X = sb.tile([64, NWIDE], f32)
        W64r = sb.tile([8, 64], f32r)
        nc.sync.dma_start(X[0:32, :], xa_ext[:])
        nc.scalar.dma_start(X[32:64, :], xb_ext[:])
        nc.sync.dma_start(W64r[:], w64_ext[:])
        thEO = X[:, 2 * NIN : 2 * NIN + 2]  # theta[2k], theta[2k+1] per row

        # ---- gpsimd: the one mask it builds ----
        # W[p, 40s+8f+b] = +/- delta(p//8, b); box2 rows 0:8 (+,+ taps),
        # alt2 rows 32:40 (+ tap0, - tap1). Dead bands f=1..3 never read.
        fill0 = nc.gpsimd.to_reg(0.0)
        W = sb.tile([64, 80], f32)
        nc.gpsimd.memset(W[:], 1.0)
        nc.gpsimd.affine_select(
            out=W[:], in_=W[:], compare_op=ALU.is_ge, fill=fill0,
            base=0, pattern=[[0, 10], [-8, 8]], channel_multiplier=1,
        )
        nc.gpsimd.affine_select(
            out=W[:], in_=W[:], compare_op=ALU.is_ge, fill=fill0,
            base=7, pattern=[[0, 10], [8, 8]], channel_multiplier=-1,
        )
        nc.gpsimd.tensor_scalar_mul(W[:, 72:80], W[:, 72:80], -1.0)

        # ---- ACT table prefetch (Square/Copy family) in DMA dead time ----
        dz = sb.tile([1, 2], f32)
        nc.vector.memset(dz[:], 1.0)
        dzo = sb.tile([1, 2], f32)
        act_raw(dzo[:], dz[:], AF.Square)

        # ---- vector: f32r weight cast, then squares ----
        Wr = sb.tile([64, 80], f32r)
        nc.vector.tensor_copy(Wr[:], W[:])
        xsqA = sb.tile([64, NIN], f32r)
        nc.vector.tensor_mul(xsqA[:], X[:, 0:NIN], X[:, 0:NIN])
        xsqB = sb.tile([64, NIN], f32r)
        nc.vector.tensor_mul(xsqB[:], X[:, NIN : 2 * NIN], X[:, NIN : 2 * NIN])

        # ---- scalar: cos(thetaEO) -> csEO [64, 2] ----
        q0 = sb.tile([64, 2], f32)
        act_raw(q0[:], thEO, AF.Square)
        g1 = sb.tile([64, 2], f32)
        act_raw(g1[:], q0[:], AF.Square, bias=CQ[1], scale=CQ[0])
        g2 = sb.tile([64, 2], f32)
        act_raw(g2[:], g1[:], AF.Square, bias=CQ[3], scale=CQ[2])
        g3 = sb.tile([64, 2], f32)
        act_raw(g3[:], g2[:], AF.Square, bias=CQ[5], scale=CQ[4])
        csEO = sb.tile([64, 2], f32)
        act_raw(csEO[:], g3[:], AF.Copy, bias=CQ[7], scale=CQ[6])

        # ---- box2/alt2 via 4 accumulating matmuls ----
        ba_ps = psp.tile([40, NIN - 1], f32, tag="ba")
        nc.tensor.matmul(ba_ps[:], Wr[:, 0:40], xsqA[:, 0 : NIN - 1],
                         start=True, stop=False)
        nc.tensor.matmul(ba_ps[:], Wr[:, 40:80], xsqA[:, 1:NIN],
                         start=False, stop=False)
        nc.tensor.matmul(ba_ps[:], Wr[:, 0:40], xsqB[:, 0 : NIN - 1],
                         start=False, stop=False)
        nc.tensor.matmul(ba_ps[:], Wr[:, 40:80], xsqB[:, 1:NIN],
                         start=False, stop=True)

        # ---- filters ----
        BA2 = sb.tile([40, NIN - 1], f32)
        nc.vector.tensor_copy(BA2[:, 0 : NIN - 3], ba_ps[:, 0 : NIN - 3])
        BA4 = sb.tile([40, NIN - 3], f32)
        nc.vector.tensor_add(BA4[:], BA2[:, 0 : NIN - 3], ba_ps[:, 2 : NIN - 1])
        B8 = sb.tile([8, NOUT], f32)
        nc.vector.tensor_add(B8[:], BA4[0:8, 0:NOUT], BA4[0:8, 4 : NIN - 3])
        A8 = sb.tile([8, NOUT], f32)
        nc.vector.tensor_add(A8[:], BA4[32:40, 0:NOUT], BA4[32:40, 4 : NIN - 3])
        inv = sb.tile([8, NOUT], f32)
        nc.vector.reciprocal_approx_fast(inv[:], B8[:])
        zn = sb.tile([8, NOUT], f32r)
        nc.vector.tensor_mul(zn[:], A8[:], inv[:])

        # ---- broadcast matmuls + parallel scaled evacuation ----
        # two PSUM tiles (the 2nd matmul is ~115ns marginal on the idle
        # PE) so the vector and scalar evacs don't serialize on a shared
        # PSUM-tile dependency; DMAs are column halves.
        opE_ps = psp.tile([64, NOUT], f32, tag="opE")
        nc.tensor.matmul(opE_ps[:], W64r[:], zn[:], start=True, stop=True)
        opO_ps = psp.tile([64, NOUT], f32, tag="opO")
        nc.tensor.matmul(opO_ps[:], W64r[:], zn[:], start=True, stop=True)
        outsE = sb.tile([64, NOUT], f32)
        nc.vector.tensor_scalar(
            outsE[:], opE_ps[:], csEO[:, 0:1], None, op0=ALU.mult
        )
        nc.sync.dma_start(out_ext[:, 0:NOUT], outsE[:])
        outsO = sb.tile([64, NOUT], f32)
        nc.scalar.activation(outsO[:], opO_ps[:], AF.Copy, scale=csEO[:, 1:2])
        nc.scalar.dma_start(out_ext[:, NOUT : 2 * NOUT], outsO[:])

    nc.compile()
    return nc


def _make_in_maps(x, theta):
    """Core k gets all batches for output cols [128k, 128k+128) as a
    [64, 272] block (rows b*8+c2; cols: even half, odd half, theta[2k],
    theta[2k+1]), one-padded past the end of x."""
    xpad = np.ones((B, C_IN, NCORE * NOUT + K - 1), dtype=np.float32)
    xpad[:, :, :L] = x
    thE = np.tile(theta[0::2, 0], B).astype(np.float32)   # [64] = theta[2k]
    thO = np.tile(theta[1::2, 0], B).astype(np.float32)   # [64] = theta[2k+1]
    p = np.arange(64)
    w64 = (np.arange(8)[:, None] == (p // 8)[None, :]).astype(np.float32)
    in_maps = []
    for k in range(NCORE):
        lo = k * NOUT
        blk = xpad[:, :, lo : lo + NIN].reshape(B, 8, 2, NIN)
        xs = np.empty((64, NWIDE), dtype=np.float32)
        xs[:, 0:NIN] = blk[:, :, 0, :].reshape(64, NIN)
        xs[:, NIN : 2 * NIN] = blk[:, :, 1, :].reshape(64, NIN)
        xs[:, 2 * NIN] = thE
        xs[:, 2 * NIN + 1] = thO
        in_maps.append({
            "xa": np.ascontiguousarray(xs[0:32]),
            "xb": np.ascontiguousarray(xs[32:64]),
            "w64": w64,
        })
    return in_maps


def _assemble(results):
    """res["out"] is [64, 256]: row 8b+k, col 128h+t -> out[b, 2k+h, t]."""
    out = np.empty((B, C_OUT, L_OUT), dtype=np.float32)
    for k in range(NCORE):
        lo = k * NOUT
        nk = min(NOUT, L_OUT - lo)
        blk = results[k]["out"].reshape(B, 8, 2, NOUT).reshape(B, C_OUT, NOUT)
        out[:, :, lo : lo + nk] = blk[:, :, :nk]
    return out


def kernel(**inputs):
    from concourse.bass_utils import run_bass_kernel_spmd

    x = np.ascontiguousarray(np.asarray(inputs["x"], dtype=np.float32))
    theta = np.ascontiguousarray(np.asarray(inputs["theta"], dtype=np.float32))

    if "nc" not in _CACHE:
        _CACHE["nc"] = _build_nc()
    nc = _CACHE["nc"]

    in_maps = _make_in_maps(x, theta)
    res = run_bass_kernel_spmd(nc, in_maps, core_ids=list(range(NCORE)))
    return _assemble(res.results)
